# revision 20
# baseline (speedup 1.0000x reference)
"""LIF (leaky integrate-and-fire) scan kernel for Trainium2, 8 NeuronCores.

Reference semantics (fp32, T=8 innermost axis):
    mem = 0
    for t in range(T):
        mem = mem * 0.5 + x[..., t]
        s[..., t] = (mem >= 1.0)
        mem = mem * (1.0 - s[..., t])

Sharding: data-parallel over the leading dim (64 -> 8 batches per core).

Memory-roofline design: the input must stream 32 MiB/core of fp32, but
the output is binary, so it leaves the device as uint8 (8 MiB/core
instead of 32 MiB).  The host maps spikes back with (y == 1) -> f32,
immune to whether the device's f32->u8 conversion saturates or wraps.

Per-core layout is chunk-major/t-minor: x[p, (c*T + t)*CH + n], so every
strip either engine touches is unit-stride.

Op placement (measured: DVE stt 2.28us, Act 2.0us per [128,2048] strip;
CH=2048 amortizes the ~150ns fixed per-instruction cost): the serial
recurrence A -> C -> A stays entirely on the DVE --
    A: m = (r mult 0.5) add x_t        scalar_tensor_tensor
    C: r = (m is_lt 1) mult m          scalar_tensor_tensor
with chunks interleaved t-outer/chunk-inner inside groups of 2, so the
in-order DVE stream always has the other chunk's op between a chunk's C
and its next A: the chain never stalls (measured: <1us of DVE gaps over
the whole run).  Only the terminal spike op
    B: y_t = Sign(m - 1) -> uint8      activation
hangs off-chain on the Act engine (the very last spike runs on the
then-idle DVE to shorten the drain tail).  Alternatives measured and
rejected: Pool stt is unsupported, Pool tensor_scalar runs at 16.5us,
Pool tensor_tensor (3.2us) stalls the chain when the reset is offloaded
(v5b: +53us), and PE identity-matmul updates need an on-chain
PSUM->SBUF copy (v3: every engine ~50% latency-stalled).

DMA: input strips ride the qSP hardware DGE queue only -- the SP
sequencer runs no compute, so strips always stream ahead of the DVE
(101us supply vs ~128us demand); routing any strips via qAct stalled the
last group by 17us (Act issues them only after the prior group's B ops).
Spike strips are written back per-timestep on the software qPool queue
the moment each B completes, so the drain tail after the last spike is a
single 2 KiB/partition transfer.

Measured on 8-core trn2: 144.6-147.1us (run-to-run device-clock
variance ~5%) vs the 219.9us fp32-roundtrip baseline; bitwise-exact
output.
"""

import numpy as np

import concourse.bass as bass
import concourse.tile as tile
from concourse import bacc, mybir
from concourse.bass_utils import run_bass_kernel_spmd

P = 128           # SBUF partitions
T = 8             # timesteps (innermost axis of the original input)
NPB = 8192        # neurons per partition per core: 8*128*32*32 / 128
CH = 2048         # neurons per chunk (per partition)
NCH = NPB // CH   # 8 chunks
GROUPS = [[0, 1], [2, 3]]

THRESH = 1.0
DECAY = 0.5
F32 = mybir.dt.float32
U8 = mybir.dt.uint8
N_CORES = 8

Alu = mybir.AluOpType
Act = mybir.ActivationFunctionType


def _build() -> bass.Bass:
    nc = bacc.Bacc("TRN2", target_bir_lowering=False, debug=False)
    x = nc.dram_tensor("x", [P, NCH * T * CH], F32, kind="ExternalInput").ap()
    y = nc.dram_tensor("y", [P, NCH * T * CH], U8, kind="ExternalOutput").ap()

    HALF = CH // 2

    with tile.TileContext(nc) as tc:
        with (
            tc.tile_pool(name="consts", bufs=1) as cpool,
            tc.tile_pool(name="xs", bufs=16) as xpool,
            tc.tile_pool(name="ys", bufs=8) as ypool,
            tc.tile_pool(name="ms", bufs=3) as mpool,
            tc.tile_pool(name="rs", bufs=3) as rpool,
            tc.tile_pool(name="gates", bufs=3) as gpool,
        ):
            neg_thresh = cpool.tile([P, 1], F32, tag="negth", name="neg_thresh")
            nc.gpsimd.memset(neg_thresh[:], -float(THRESH))

            def spike(c, t, m_ap, on_dve=False):
                # B: u8 spike strip, written straight out on qPool.  The
                # very last spike runs on the then-idle DVE (2x-mode
                # tensor_scalar) instead of queueing behind Act's final op.
                ys = ypool.tile([P, CH], U8, tag="y", name=f"y{c}_{t}")
                if on_dve:
                    nc.vector.tensor_scalar(
                        ys[:], m_ap, THRESH, None, Alu.is_ge, Alu.bypass
                    )
                else:
                    nc.scalar.activation(
                        ys[:], m_ap, Act.Sign, bias=neg_thresh[:]
                    )
                nc.gpsimd.dma_start(
                    y[:, (c * T + t) * CH : (c * T + t + 1) * CH], ys[:]
                )

            for chunks in GROUPS:
                xs = {c: [None] * T for c in chunks}
                for t in range(T):
                    for c in chunks:
                        st = xpool.tile([P, CH], F32, tag="x", name=f"x{c}_{t}")
                        nc.sync.dma_start(
                            st[:], x[:, (c * T + t) * CH : (c * T + t + 1) * CH]
                        )
                        xs[c][t] = st

                r = {}
                for c in chunks:
                    r[c] = rpool.tile([P, CH], F32, tag="r", name=f"r{c}")

                def reset_gate(c, t, src_ap):
                    # DVE 2x tensor_scalar gate for the Pool half of the
                    # reset, issued immediately after the A that feeds it so
                    # the Pool mult lands well before the next round's A.
                    g = gpool.tile([P, HALF], F32, tag="g", name=f"g{c}_{t}")
                    nc.vector.tensor_scalar(
                        g[:], src_ap[:, HALF:CH], THRESH, None,
                        Alu.is_lt, Alu.bypass,
                    )
                    return g

                def reset_pool_half(c, g, src_ap):
                    nc.gpsimd.tensor_tensor(
                        r[c][:, HALF:CH], g[:], src_ap[:, HALF:CH], Alu.mult
                    )

                def reset_dve_half(c, src_ap):
                    nc.vector.scalar_tensor_tensor(
                        r[c][:, 0:HALF], src_ap[:, 0:HALF], THRESH,
                        src_ap[:, 0:HALF], Alu.is_lt, Alu.mult,
                    )

                # t = 0: mem0 = 0 so m == x_0 (read the strip directly).
                g = {}
                for c in chunks:
                    g[c] = reset_gate(c, 0, xs[c][0][:])
                for c in chunks:
                    reset_pool_half(c, g[c], xs[c][0][:])
                for c in chunks:
                    spike(c, 0, xs[c][0][:])
                for c in chunks:
                    reset_dve_half(c, xs[c][0][:])

                for t in range(1, T):
                    m, g = {}, {}
                    for c in chunks:
                        ms = mpool.tile([P, CH], F32, tag="m", name=f"m{c}_{t}")
                        nc.vector.scalar_tensor_tensor(
                            ms[:], r[c][:], DECAY, xs[c][t][:],
                            Alu.mult, Alu.add,
                        )
                        m[c] = ms
                        if t < T - 1:
                            g[c] = reset_gate(c, t, ms[:])
                    if t < T - 1:
                        for c in chunks:
                            reset_pool_half(c, g[c], m[c][:])
                    last = chunks is GROUPS[-1] and t == T - 1
                    for c in chunks:
                        spike(c, t, m[c][:], on_dve=last and c == chunks[-1])
                    if t < T - 1:
                        for c in chunks:
                            reset_dve_half(c, m[c][:])
    nc.compile()
    return nc


_NC_CACHE: bass.Bass | None = None


def _get_nc() -> bass.Bass:
    global _NC_CACHE
    if _NC_CACHE is None:
        _NC_CACHE = _build()
    return _NC_CACHE


def _run(X: np.ndarray, **spmd_kwargs):
    assert X.shape == (64, 128, 32, 32, 8), X.shape
    X = np.ascontiguousarray(X, dtype=np.float32)
    per_core = 64 // N_CORES
    # [core, p, nch, ch, t] -> chunk-major t-minor [core, p, nch, t, ch]
    Xt = np.ascontiguousarray(
        X.reshape(N_CORES, P, NCH, CH, T).transpose(0, 1, 2, 4, 3)
    )
    in_maps = [{"x": Xt[i].reshape(P, NCH * T * CH)} for i in range(N_CORES)]
    res = run_bass_kernel_spmd(
        _get_nc(), in_maps, core_ids=list(range(N_CORES)), **spmd_kwargs
    )
    out = np.empty_like(X)
    for i, rr in enumerate(res.results):
        s = rr["y"].reshape(P, NCH, T, CH).transpose(0, 1, 3, 2)
        out[i * per_core : (i + 1) * per_core] = (
            (s == 1).astype(np.float32).reshape(per_core, 128, 32, 32, 8)
        )
    return out, res


def kernel(X: np.ndarray) -> np.ndarray:
    out, _ = _run(X)
    return out


# revision 21
# speedup vs baseline: 1.0406x; 1.0406x over previous
"""LIF (leaky integrate-and-fire) scan kernel for Trainium2, 8 NeuronCores.

Reference semantics (fp32, T=8 innermost axis):
    mem = 0
    for t in range(T):
        mem = mem * 0.5 + x[..., t]
        s[..., t] = (mem >= 1.0)
        mem = mem * (1.0 - s[..., t])

Sharding: data-parallel over the leading dim (64 -> 8 batches per core).

Memory-roofline design: the input must stream 32 MiB/core of fp32, but
the output is binary, so it leaves the device as uint8 (8 MiB/core
instead of 32 MiB).  The host maps spikes back with (y == 1) -> f32,
immune to whether the device's f32->u8 conversion saturates or wraps.

Per-core layout is chunk-major/t-minor: x[p, (c*T + t)*CH + n], so every
strip either engine touches is unit-stride.

Op placement (measured: DVE stt 2.28us, Act 2.0us per [128,2048] strip;
CH=2048 amortizes the ~150ns fixed per-instruction cost): the serial
recurrence A -> C -> A stays entirely on the DVE --
    A: m = (r mult 0.5) add x_t        scalar_tensor_tensor
    C: r = (m is_lt 1) mult m          scalar_tensor_tensor
with chunks interleaved t-outer/chunk-inner inside groups of 2, so the
in-order DVE stream always has the other chunk's op between a chunk's C
and its next A: the chain never stalls (measured: <1us of DVE gaps over
the whole run).  Only the terminal spike op
    B: y_t = Sign(m - 1) -> uint8      activation
hangs off-chain on the Act engine (the very last spike runs on the
then-idle DVE to shorten the drain tail).  Alternatives measured and
rejected: Pool stt is unsupported, Pool tensor_scalar runs at 16.5us,
Pool tensor_tensor (3.2us) stalls the chain when the reset is offloaded
(v5b: +53us), and PE identity-matmul updates need an on-chain
PSUM->SBUF copy (v3: every engine ~50% latency-stalled).

DMA: input strips ride the qSP hardware DGE queue only -- the SP
sequencer runs no compute, so strips always stream ahead of the DVE
(101us supply vs ~128us demand); routing any strips via qAct stalled the
last group by 17us (Act issues them only after the prior group's B ops).
Spike strips are written back per-timestep on the software qPool queue
the moment each B completes, so the drain tail after the last spike is a
single 2 KiB/partition transfer.

Measured on 8-core trn2: 144.6-147.1us (run-to-run device-clock
variance ~5%) vs the 219.9us fp32-roundtrip baseline; bitwise-exact
output.
"""

import numpy as np

import concourse.bass as bass
import concourse.tile as tile
from concourse import bacc, mybir
from concourse.bass_utils import run_bass_kernel_spmd

P = 128           # SBUF partitions
T = 8             # timesteps (innermost axis of the original input)
NPB = 8192        # neurons per partition per core: 8*128*32*32 / 128
CH = 2048         # neurons per chunk (per partition)
NCH = NPB // CH   # 8 chunks
GROUPS = [[0, 1], [2, 3]]

THRESH = 1.0
DECAY = 0.5
F32 = mybir.dt.float32
U8 = mybir.dt.uint8
N_CORES = 8

Alu = mybir.AluOpType
Act = mybir.ActivationFunctionType


def _build() -> bass.Bass:
    nc = bacc.Bacc("TRN2", target_bir_lowering=False, debug=False)
    x = nc.dram_tensor("x", [P, NCH * T * CH], F32, kind="ExternalInput").ap()
    y = nc.dram_tensor("y", [P, NCH * T * CH], U8, kind="ExternalOutput").ap()

    HALF = CH // 2

    with tile.TileContext(nc) as tc:
        with (
            tc.tile_pool(name="consts", bufs=1) as cpool,
            tc.tile_pool(name="xs", bufs=16) as xpool,
            tc.tile_pool(name="ys", bufs=8) as ypool,
            tc.tile_pool(name="ms", bufs=3) as mpool,
            tc.tile_pool(name="rs", bufs=3) as rpool,
            tc.tile_pool(name="gates", bufs=3) as gpool,
        ):
            neg_thresh = cpool.tile([P, 1], F32, tag="negth", name="neg_thresh")
            nc.gpsimd.memset(neg_thresh[:], -float(THRESH))

            def spike(c, t, m_ap, on_dve=False):
                # B: u8 spike strip, written straight out on qPool.  The
                # very last spike runs on the then-idle DVE (2x-mode
                # tensor_scalar) instead of queueing behind Act's final op.
                ys = ypool.tile([P, CH], U8, tag="y", name=f"y{c}_{t}")
                if on_dve:
                    nc.vector.tensor_scalar(
                        ys[:], m_ap, THRESH, None, Alu.is_ge, Alu.bypass
                    )
                else:
                    nc.scalar.activation(
                        ys[:], m_ap, Act.Sign, bias=neg_thresh[:]
                    )
                # Issue on qAct: the issue follows its producer B on the same
                # sequencer, so it never head-of-line-blocks another engine
                # (on gpsimd it stalled the Pool reset mults behind Act).
                nc.scalar.dma_start(
                    y[:, (c * T + t) * CH : (c * T + t + 1) * CH], ys[:]
                )

            for chunks in GROUPS:
                xs = {c: [None] * T for c in chunks}
                for t in range(T):
                    for c in chunks:
                        st = xpool.tile([P, CH], F32, tag="x", name=f"x{c}_{t}")
                        nc.sync.dma_start(
                            st[:], x[:, (c * T + t) * CH : (c * T + t + 1) * CH]
                        )
                        xs[c][t] = st

                r = {}
                for c in chunks:
                    r[c] = rpool.tile([P, CH], F32, tag="r", name=f"r{c}")

                def reset_gate(c, t, src_ap):
                    # DVE 2x tensor_scalar gate for the Pool half of the
                    # reset, issued immediately after the A that feeds it so
                    # the Pool mult lands well before the next round's A.
                    g = gpool.tile([P, HALF], F32, tag="g", name=f"g{c}_{t}")
                    nc.vector.tensor_scalar(
                        g[:], src_ap[:, HALF:CH], THRESH, None,
                        Alu.is_lt, Alu.bypass,
                    )
                    return g

                def reset_pool_half(c, g, src_ap):
                    nc.gpsimd.tensor_tensor(
                        r[c][:, HALF:CH], g[:], src_ap[:, HALF:CH], Alu.mult
                    )

                def reset_dve_half(c, src_ap):
                    nc.vector.scalar_tensor_tensor(
                        r[c][:, 0:HALF], src_ap[:, 0:HALF], THRESH,
                        src_ap[:, 0:HALF], Alu.is_lt, Alu.mult,
                    )

                # t = 0: mem0 = 0 so m == x_0 (read the strip directly).
                g = {}
                for c in chunks:
                    g[c] = reset_gate(c, 0, xs[c][0][:])
                for c in chunks:
                    reset_pool_half(c, g[c], xs[c][0][:])
                for c in chunks:
                    spike(c, 0, xs[c][0][:])
                for c in chunks:
                    reset_dve_half(c, xs[c][0][:])

                for t in range(1, T):
                    m, g = {}, {}
                    for c in chunks:
                        ms = mpool.tile([P, CH], F32, tag="m", name=f"m{c}_{t}")
                        nc.vector.scalar_tensor_tensor(
                            ms[:], r[c][:], DECAY, xs[c][t][:],
                            Alu.mult, Alu.add,
                        )
                        m[c] = ms
                        if t < T - 1:
                            g[c] = reset_gate(c, t, ms[:])
                    if t < T - 1:
                        for c in chunks:
                            reset_pool_half(c, g[c], m[c][:])
                    last = chunks is GROUPS[-1] and t == T - 1
                    for c in chunks:
                        spike(c, t, m[c][:], on_dve=last and c == chunks[-1])
                    if t < T - 1:
                        for c in chunks:
                            reset_dve_half(c, m[c][:])
    nc.compile()
    return nc


_NC_CACHE: bass.Bass | None = None


def _get_nc() -> bass.Bass:
    global _NC_CACHE
    if _NC_CACHE is None:
        _NC_CACHE = _build()
    return _NC_CACHE


def _run(X: np.ndarray, **spmd_kwargs):
    assert X.shape == (64, 128, 32, 32, 8), X.shape
    X = np.ascontiguousarray(X, dtype=np.float32)
    per_core = 64 // N_CORES
    # [core, p, nch, ch, t] -> chunk-major t-minor [core, p, nch, t, ch]
    Xt = np.ascontiguousarray(
        X.reshape(N_CORES, P, NCH, CH, T).transpose(0, 1, 2, 4, 3)
    )
    in_maps = [{"x": Xt[i].reshape(P, NCH * T * CH)} for i in range(N_CORES)]
    res = run_bass_kernel_spmd(
        _get_nc(), in_maps, core_ids=list(range(N_CORES)), **spmd_kwargs
    )
    out = np.empty_like(X)
    for i, rr in enumerate(res.results):
        s = rr["y"].reshape(P, NCH, T, CH).transpose(0, 1, 3, 2)
        out[i * per_core : (i + 1) * per_core] = (
            (s == 1).astype(np.float32).reshape(per_core, 128, 32, 32, 8)
        )
    return out, res


def kernel(X: np.ndarray) -> np.ndarray:
    out, _ = _run(X)
    return out


# revision 22
# speedup vs baseline: 1.3134x; 1.2622x over previous
"""LIF (leaky integrate-and-fire) scan kernel for Trainium2, 8 NeuronCores.

Reference semantics (fp32, T=8 innermost axis):
    mem = 0
    for t in range(T):
        mem = mem * 0.5 + x[..., t]
        s[..., t] = (mem >= 1.0)
        mem = mem * (1.0 - s[..., t])

Sharding: data-parallel over the leading dim (64 -> 8 batches per core).

Memory-roofline design: the input must stream 32 MiB/core of fp32, but
the output is binary, so it leaves the device as uint8 (8 MiB/core
instead of 32 MiB).  The host maps spikes back with (y == 1) -> f32,
immune to whether the device's f32->u8 conversion saturates or wraps.

Per-core layout is chunk-major/t-minor: x[p, (c*T + t)*CH + n], so every
strip either engine touches is unit-stride.

Op placement (measured: DVE stt 2.28us, Act 2.0us per [128,2048] strip;
CH=2048 amortizes the ~150ns fixed per-instruction cost): the serial
recurrence A -> C -> A stays entirely on the DVE --
    A: m = (r mult 0.5) add x_t        scalar_tensor_tensor
    C: r = (m is_lt 1) mult m          scalar_tensor_tensor
with chunks interleaved t-outer/chunk-inner inside groups of 2, so the
in-order DVE stream always has the other chunk's op between a chunk's C
and its next A: the chain never stalls (measured: <1us of DVE gaps over
the whole run).  Only the terminal spike op
    B: y_t = Sign(m - 1) -> uint8      activation
hangs off-chain on the Act engine (the very last spike runs on the
then-idle DVE to shorten the drain tail).  Alternatives measured and
rejected: Pool stt is unsupported, Pool tensor_scalar runs at 16.5us,
Pool tensor_tensor (3.2us) stalls the chain when the reset is offloaded
(v5b: +53us), and PE identity-matmul updates need an on-chain
PSUM->SBUF copy (v3: every engine ~50% latency-stalled).

DMA: input strips ride the qSP hardware DGE queue only -- the SP
sequencer runs no compute, so strips always stream ahead of the DVE
(101us supply vs ~128us demand); routing any strips via qAct stalled the
last group by 17us (Act issues them only after the prior group's B ops).
Spike strips are written back per-timestep on the software qPool queue
the moment each B completes, so the drain tail after the last spike is a
single 2 KiB/partition transfer.

Measured on 8-core trn2: 144.6-147.1us (run-to-run device-clock
variance ~5%) vs the 219.9us fp32-roundtrip baseline; bitwise-exact
output.
"""

import numpy as np

import concourse.bass as bass
import concourse.tile as tile
from concourse import bacc, mybir
from concourse.bass_utils import run_bass_kernel_spmd

P = 128           # SBUF partitions
T = 8             # timesteps (innermost axis of the original input)
NPB = 8192        # neurons per partition per core: 8*128*32*32 / 128
CH = 2048         # neurons per chunk (per partition)
NCH = NPB // CH   # 8 chunks
GROUPS = [[0, 1], [2, 3]]

THRESH = 1.0
DECAY = 0.5
F32 = mybir.dt.float32
U8 = mybir.dt.uint8
N_CORES = 8

Alu = mybir.AluOpType
Act = mybir.ActivationFunctionType


def _build() -> bass.Bass:
    nc = bacc.Bacc("TRN2", target_bir_lowering=False, debug=False)
    x = nc.dram_tensor("x", [P, NCH * T * CH], F32, kind="ExternalInput").ap()
    y = nc.dram_tensor("y", [P, NCH * T * CH], U8, kind="ExternalOutput").ap()

    HALF = CH // 2

    with tile.TileContext(nc) as tc:
        with (
            tc.tile_pool(name="consts", bufs=1) as cpool,
            tc.tile_pool(name="xs", bufs=16) as xpool,
            tc.tile_pool(name="ys", bufs=8) as ypool,
            tc.tile_pool(name="ms", bufs=3) as mpool,
            tc.tile_pool(name="rs", bufs=3) as rpool,
            tc.tile_pool(name="gates", bufs=3) as gpool,
        ):
            neg_thresh = cpool.tile([P, 1], F32, tag="negth", name="neg_thresh")
            nc.gpsimd.memset(neg_thresh[:], -float(THRESH))

            def spike(c, t, m_ap, on_dve=False):
                # B: u8 spike strip, written straight out on qPool.  The
                # very last spike runs on the then-idle DVE (2x-mode
                # tensor_scalar) instead of queueing behind Act's final op.
                ys = ypool.tile([P, CH], U8, tag="y", name=f"y{c}_{t}")
                if on_dve:
                    nc.vector.tensor_scalar(
                        ys[:], m_ap, THRESH, None, Alu.is_ge, Alu.bypass
                    )
                else:
                    nc.scalar.activation(
                        ys[:], m_ap, Act.Sign, bias=neg_thresh[:]
                    )
                # Issue on qAct: the issue follows its producer B on the same
                # sequencer, so it never head-of-line-blocks another engine
                # (on gpsimd it stalled the Pool reset mults behind Act).
                nc.scalar.dma_start(
                    y[:, (c * T + t) * CH : (c * T + t + 1) * CH], ys[:]
                )

            for chunks in GROUPS:
                xs = {c: [None] * T for c in chunks}
                for t in range(T):
                    for c in chunks:
                        st = xpool.tile([P, CH], F32, tag="x", name=f"x{c}_{t}")
                        nc.sync.dma_start(
                            st[:], x[:, (c * T + t) * CH : (c * T + t + 1) * CH]
                        )
                        xs[c][t] = st

                r = {}
                for c in chunks:
                    r[c] = rpool.tile([P, CH], F32, tag="r", name=f"r{c}")

                # Full-width reset on the DVE.  Splitting half of it to the
                # Pool engine (gate + tensor_tensor mult) was measured twice:
                # the Pool op's 2.4-3.2us execution plus two semaphore hops
                # never lands before the next round's A, so the chain stalls
                # (+45-53us).  The recurrence stays DVE-only.

                # t = 0: mem0 = 0 so m == x_0 (read the strip directly).
                for c in chunks:
                    spike(c, 0, xs[c][0][:])
                for c in chunks:
                    nc.vector.scalar_tensor_tensor(
                        r[c][:], xs[c][0][:], THRESH, xs[c][0][:],
                        Alu.is_lt, Alu.mult,
                    )

                for t in range(1, T):
                    m = {}
                    for c in chunks:
                        ms = mpool.tile([P, CH], F32, tag="m", name=f"m{c}_{t}")
                        nc.vector.scalar_tensor_tensor(
                            ms[:], r[c][:], DECAY, xs[c][t][:],
                            Alu.mult, Alu.add,
                        )
                        m[c] = ms
                    last = chunks is GROUPS[-1] and t == T - 1
                    for c in chunks:
                        spike(c, t, m[c][:], on_dve=last and c == chunks[-1])
                    if t < T - 1:
                        for c in chunks:
                            nc.vector.scalar_tensor_tensor(
                                r[c][:], m[c][:], THRESH, m[c][:],
                                Alu.is_lt, Alu.mult,
                            )
    nc.compile()
    return nc


_NC_CACHE: bass.Bass | None = None


def _get_nc() -> bass.Bass:
    global _NC_CACHE
    if _NC_CACHE is None:
        _NC_CACHE = _build()
    return _NC_CACHE


def _run(X: np.ndarray, **spmd_kwargs):
    assert X.shape == (64, 128, 32, 32, 8), X.shape
    X = np.ascontiguousarray(X, dtype=np.float32)
    per_core = 64 // N_CORES
    # [core, p, nch, ch, t] -> chunk-major t-minor [core, p, nch, t, ch]
    Xt = np.ascontiguousarray(
        X.reshape(N_CORES, P, NCH, CH, T).transpose(0, 1, 2, 4, 3)
    )
    in_maps = [{"x": Xt[i].reshape(P, NCH * T * CH)} for i in range(N_CORES)]
    res = run_bass_kernel_spmd(
        _get_nc(), in_maps, core_ids=list(range(N_CORES)), **spmd_kwargs
    )
    out = np.empty_like(X)
    for i, rr in enumerate(res.results):
        s = rr["y"].reshape(P, NCH, T, CH).transpose(0, 1, 3, 2)
        out[i * per_core : (i + 1) * per_core] = (
            (s == 1).astype(np.float32).reshape(per_core, 128, 32, 32, 8)
        )
    return out, res


def kernel(X: np.ndarray) -> np.ndarray:
    out, _ = _run(X)
    return out


# revision 23
# speedup vs baseline: 1.3710x; 1.0438x over previous
"""LIF (leaky integrate-and-fire) scan kernel for Trainium2, 8 NeuronCores.

Reference semantics (fp32, T=8 innermost axis):
    mem = 0
    for t in range(T):
        mem = mem * 0.5 + x[..., t]
        s[..., t] = (mem >= 1.0)
        mem = mem * (1.0 - s[..., t])

Sharding: data-parallel over the leading dim (64 -> 8 batches per core).

Memory-roofline design: the input must stream 32 MiB/core of fp32, but
the output is binary, so it leaves the device as uint8 (8 MiB/core
instead of 32 MiB).  The host maps spikes back with (y == 1) -> f32,
immune to whether the device's f32->u8 conversion saturates or wraps.

Per-core layout is chunk-major/t-minor: x[p, (c*T + t)*CH + n], so every
strip either engine touches is unit-stride.

Op placement (measured: DVE stt 2.28us, Act 2.0us per [128,2048] strip;
CH=2048 amortizes the ~150ns fixed per-instruction cost): the serial
recurrence A -> C -> A stays entirely on the DVE --
    A: m = (r mult 0.5) add x_t        scalar_tensor_tensor
    C: r = (m is_lt 1) mult m          scalar_tensor_tensor
with chunks interleaved t-outer/chunk-inner inside groups of 2, so the
in-order DVE stream always has the other chunk's op between a chunk's C
and its next A: the chain never stalls (measured: <1us of DVE gaps over
the whole run).  Only the terminal spike op
    B: y_t = Sign(m - 1) -> uint8      activation
hangs off-chain on the Act engine (the very last spike runs on the
then-idle DVE to shorten the drain tail).  Alternatives measured and
rejected: Pool stt is unsupported, Pool tensor_scalar runs at 16.5us,
Pool tensor_tensor (3.2us) stalls the chain when the reset is offloaded
(v5b: +53us), and PE identity-matmul updates need an on-chain
PSUM->SBUF copy (v3: every engine ~50% latency-stalled).

DMA: input strips ride the qSP hardware DGE queue only -- the SP
sequencer runs no compute, so strips always stream ahead of the DVE
(101us supply vs ~128us demand); routing any strips via qAct stalled the
last group by 17us (Act issues them only after the prior group's B ops).
Spike strips are written back per-timestep on the software qPool queue
the moment each B completes, so the drain tail after the last spike is a
single 2 KiB/partition transfer.

Measured on 8-core trn2: 144.6-147.1us (run-to-run device-clock
variance ~5%) vs the 219.9us fp32-roundtrip baseline; bitwise-exact
output.
"""

import numpy as np

import concourse.bass as bass
import concourse.tile as tile
from concourse import bacc, mybir
from concourse.bass_utils import run_bass_kernel_spmd

P = 128           # SBUF partitions
T = 8             # timesteps (innermost axis of the original input)
NPB = 8192        # neurons per partition per core: 8*128*32*32 / 128
CH = 2048         # neurons per chunk (per partition)
NCH = NPB // CH   # 8 chunks
GROUPS = [[0, 1], [2, 3]]

THRESH = 1.0
DECAY = 0.5
F32 = mybir.dt.float32
U8 = mybir.dt.uint8
N_CORES = 8

Alu = mybir.AluOpType
Act = mybir.ActivationFunctionType


def _build() -> bass.Bass:
    nc = bacc.Bacc("TRN2", target_bir_lowering=False, debug=False)
    x = nc.dram_tensor("x", [P, NCH * T * CH], F32, kind="ExternalInput").ap()
    y = nc.dram_tensor("y", [P, NCH * T * CH], U8, kind="ExternalOutput").ap()

    HALF = CH // 2

    with tile.TileContext(nc) as tc:
        with (
            tc.tile_pool(name="consts", bufs=1) as cpool,
            tc.tile_pool(name="xs", bufs=16) as xpool,
            tc.tile_pool(name="ys", bufs=8) as ypool,
            tc.tile_pool(name="ms", bufs=3) as mpool,
            tc.tile_pool(name="rs", bufs=3) as rpool,
            tc.tile_pool(name="gates", bufs=3) as gpool,
        ):
            neg_thresh = cpool.tile([P, 1], F32, tag="negth", name="neg_thresh")
            nc.gpsimd.memset(neg_thresh[:], -float(THRESH))

            def spike(c, t, m_ap, on_dve=False):
                # B: u8 spike strip, written straight out on qPool.  The
                # very last spike runs on the then-idle DVE (2x-mode
                # tensor_scalar) instead of queueing behind Act's final op.
                ys = ypool.tile([P, CH], U8, tag="y", name=f"y{c}_{t}")
                if on_dve:
                    nc.vector.tensor_scalar(
                        ys[:], m_ap, THRESH, None, Alu.is_ge, Alu.bypass
                    )
                else:
                    nc.scalar.activation(
                        ys[:], m_ap, Act.Sign, bias=neg_thresh[:]
                    )
                # qPool software queue: gpsimd runs no compute here, so its
                # sequencer blocking on B-completion is harmless, and the Act
                # sequencer stays free of issue work (qAct outputs measured
                # ~6us slower end-to-end).
                nc.gpsimd.dma_start(
                    y[:, (c * T + t) * CH : (c * T + t + 1) * CH], ys[:]
                )

            for chunks in GROUPS:
                xs = {c: [None] * T for c in chunks}
                for t in range(T):
                    for c in chunks:
                        st = xpool.tile([P, CH], F32, tag="x", name=f"x{c}_{t}")
                        nc.sync.dma_start(
                            st[:], x[:, (c * T + t) * CH : (c * T + t + 1) * CH]
                        )
                        xs[c][t] = st

                r = {}
                for c in chunks:
                    r[c] = rpool.tile([P, CH], F32, tag="r", name=f"r{c}")

                # Full-width reset on the DVE.  Splitting half of it to the
                # Pool engine (gate + tensor_tensor mult) was measured twice:
                # the Pool op's 2.4-3.2us execution plus two semaphore hops
                # never lands before the next round's A, so the chain stalls
                # (+45-53us).  The recurrence stays DVE-only.

                # t = 0: mem0 = 0 so m == x_0 (read the strip directly).
                for c in chunks:
                    spike(c, 0, xs[c][0][:])
                for c in chunks:
                    nc.vector.scalar_tensor_tensor(
                        r[c][:], xs[c][0][:], THRESH, xs[c][0][:],
                        Alu.is_lt, Alu.mult,
                    )

                for t in range(1, T):
                    m = {}
                    for c in chunks:
                        ms = mpool.tile([P, CH], F32, tag="m", name=f"m{c}_{t}")
                        nc.vector.scalar_tensor_tensor(
                            ms[:], r[c][:], DECAY, xs[c][t][:],
                            Alu.mult, Alu.add,
                        )
                        m[c] = ms
                    last = chunks is GROUPS[-1] and t == T - 1
                    for c in chunks:
                        spike(c, t, m[c][:], on_dve=last and c == chunks[-1])
                    if t < T - 1:
                        for c in chunks:
                            nc.vector.scalar_tensor_tensor(
                                r[c][:], m[c][:], THRESH, m[c][:],
                                Alu.is_lt, Alu.mult,
                            )
    nc.compile()
    return nc


_NC_CACHE: bass.Bass | None = None


def _get_nc() -> bass.Bass:
    global _NC_CACHE
    if _NC_CACHE is None:
        _NC_CACHE = _build()
    return _NC_CACHE


def _run(X: np.ndarray, **spmd_kwargs):
    assert X.shape == (64, 128, 32, 32, 8), X.shape
    X = np.ascontiguousarray(X, dtype=np.float32)
    per_core = 64 // N_CORES
    # [core, p, nch, ch, t] -> chunk-major t-minor [core, p, nch, t, ch]
    Xt = np.ascontiguousarray(
        X.reshape(N_CORES, P, NCH, CH, T).transpose(0, 1, 2, 4, 3)
    )
    in_maps = [{"x": Xt[i].reshape(P, NCH * T * CH)} for i in range(N_CORES)]
    res = run_bass_kernel_spmd(
        _get_nc(), in_maps, core_ids=list(range(N_CORES)), **spmd_kwargs
    )
    out = np.empty_like(X)
    for i, rr in enumerate(res.results):
        s = rr["y"].reshape(P, NCH, T, CH).transpose(0, 1, 3, 2)
        out[i * per_core : (i + 1) * per_core] = (
            (s == 1).astype(np.float32).reshape(per_core, 128, 32, 32, 8)
        )
    return out, res


def kernel(X: np.ndarray) -> np.ndarray:
    out, _ = _run(X)
    return out
